# revision 13
# baseline (speedup 1.0000x reference)
# Multi-head attention (B=4, S=2048, E=1024, H=16) on 8 NeuronCores.
#
# Sharding: sequence-parallel. Core c handles batch b=c//2 and query rows
# [(c%2)*1024, (c%2+1)*1024) of that batch, computing all 16 heads for its
# query slice. K/V projections for the batch are computed (duplicated) on
# both cores of a pair; there are no collectives — the host concatenates
# the 8 disjoint output row-slices.
#
# Device dataflow per core (all matmuls in float32r = fp22, full PE rate):
#   inputs arrive host-pre-transposed: qT [E, 1024], kT/vT [E, 2048],
#   WqT/WkT/WvT/WoT [in, out], so every matmul has its contraction dim on
#   partitions with no on-device transposes.
#     KT  [out=8x128, k=2048]  = WkT.T @ kT + bk   (bias per-partition, DVE)
#     V'  [k, 16*(64+1)]       = (vT.T @ WvT + bv) per-head cols + ones col
#     QT  [out=8x128, q=1024]  = WqT.T @ qT + bq
#   per head pair (row-packed K=64 matmuls, heads 2j/2j+1 at array rows
#   0:64 / 64:128):
#     S.T [k=128-chunk, q=1024] = KT_h^T-slice @ QT_h-slice   (PSUM)
#     X = exp(S.T / 8)                                        (ACT, PSUM->SBUF)
#     OV'[65, q] += V'_h.T-chunk @ X   over 16 k-chunks; row 64 = softmax
#       denominator via the ones column of V'
#     resT_h [64, q] = OV'[0:64] * (1/d broadcast via K=1 ones matmul)
#   out [q, E] = sum_h resT_h.T @ WoT_h-rows + bo  -> DRAM, host concat.

import numpy as np

EMB = 1024
HEADS = 16
HD = 64
B = 4
S = 2048
NCORES = 8
P = 128
TQ = 1024  # query tokens per core
TK = 2048  # key tokens per core (= S of its batch)
OC = EMB // P  # 8 column chunks of the projection output
N = 512  # matmul moving free dim (fp32 max)
KC = TK // P  # 16 k-chunks
HP = HEADS // 2  # 8 head pairs

_CACHE = {}


def _build():
    import concourse.bass as bass  # noqa: F401
    import concourse.mybir as mybir
    from concourse import bacc
    from concourse.tile import TileContext

    f32 = mybir.dt.float32
    f32r = mybir.dt.float32r
    EXP = mybir.ActivationFunctionType.Exp

    nc = bacc.Bacc()

    qT = nc.declare_dram_parameter("qT", [EMB, TQ], f32r, isOutput=False)
    kT = nc.declare_dram_parameter("kT", [EMB, TK], f32r, isOutput=False)
    vT = nc.declare_dram_parameter("vT", [EMB, TK], f32r, isOutput=False)
    WqT = nc.declare_dram_parameter("WqT", [EMB, EMB], f32r, isOutput=False)
    WkT = nc.declare_dram_parameter("WkT", [EMB, EMB], f32r, isOutput=False)
    WvT = nc.declare_dram_parameter("WvT", [EMB, EMB], f32r, isOutput=False)
    WoT = nc.declare_dram_parameter("WoT", [EMB, EMB], f32r, isOutput=False)
    bq = nc.declare_dram_parameter("bq", [OC, P], f32, isOutput=False)
    bk = nc.declare_dram_parameter("bk", [OC, P], f32, isOutput=False)
    bv = nc.declare_dram_parameter("bv", [1, EMB], f32r, isOutput=False)
    bo = nc.declare_dram_parameter("bo", [1, EMB], f32r, isOutput=False)
    ones_d = nc.declare_dram_parameter("ones_d", [P, P], f32r, isOutput=False)
    out = nc.declare_dram_parameter("out", [TQ, EMB], f32, isOutput=True)

    # DRAM intermediates (SBUF can't hold KT + V' + QT + weights at once)
    KT_d = nc.dram_tensor("KT_d", [OC, P, TK], f32r)
    # V' grouped by head pair: [k-chunk, p, pair, 130] (65 cols per head)
    Vp_d = nc.dram_tensor("Vp_d", [KC, P, HP, 130], f32r)

    with nc.allow_low_precision(reason="fp32r (fp22) matmul pipeline by design"), TileContext(nc) as tc:
        with (
            tc.tile_pool(name="const", bufs=1) as const_pool,
            tc.tile_pool(name="res", bufs=1) as res_pool,
            tc.tile_pool(name="qtp", bufs=1) as qt_pool,
        ):
            bq_sb = const_pool.tile([P, OC], f32)
            nc.sync.dma_start(bq_sb[:], bq.rearrange("o p -> p o"))
            bk_sb = const_pool.tile([P, OC], f32)
            nc.sync.dma_start(bk_sb[:], bk.rearrange("o p -> p o"))
            bv_sb = const_pool.tile([1, EMB], f32r)
            nc.sync.dma_start(bv_sb[:], bv[:])
            bo_sb = const_pool.tile([1, EMB], f32r)
            nc.sync.dma_start(bo_sb[:], bo[:])
            ones_sb = const_pool.tile([1, P], f32r)
            nc.sync.dma_start(ones_sb[:], ones_d[0:1, :])
            # ones row living at partition 64, matching rd's base partition
            ones64_sb = const_pool.tile([65, HD], f32r)
            nc.sync.dma_start(ones64_sb[64:65, :], ones_d[0:1, 0:HD])

            qt_big = qt_pool.tile([P, OC, TQ], f32r)
            resT = [res_pool.tile([HD, TQ], f32r, tag=f"res{h}", name=f"resT{h}")
                    for h in range(HEADS)]

            # ---------------- K projection -> KT_d ----------------
            with (
                tc.tile_pool(name="wkp", bufs=1) as wpool,
                tc.tile_pool(name="kin", bufs=2) as inpool,
                tc.tile_pool(name="kout", bufs=3) as outpool,
                tc.tile_pool(name="pjps", bufs=2, space="PSUM") as pjps,
            ):
                wk_sb = wpool.tile([P, OC, EMB], f32r)
                nc.sync.dma_start(wk_sb[:], WkT.rearrange("(e p) o -> p e o", p=P))
                kT3 = kT.rearrange("(e p) t -> p e t", p=P)
                for tt in range(TK // N):
                    kin = inpool.tile([P, OC, N], f32r, tag="kin")
                    nc.sync.dma_start(kin[:], kT3[:, :, tt * N:(tt + 1) * N])
                    for oc in range(OC):
                        ps = pjps.tile([P, N], f32, tag="pj", name="ps_k")
                        for ec in range(OC):
                            nc.tensor.matmul(
                                ps[:],
                                wk_sb[:, ec, oc * P:(oc + 1) * P],
                                kin[:, ec, :],
                                start=(ec == 0),
                                stop=(ec == OC - 1),
                            )
                        kt_out = outpool.tile([P, N], f32r, tag="kout", name="kt_out")
                        nc.vector.tensor_scalar_add(kt_out[:], ps[:], bk_sb[:, oc:oc + 1])
                        nc.sync.dma_start(KT_d[oc, :, tt * N:(tt + 1) * N], kt_out[:])

            # ---------------- V projection -> Vp_d ----------------
            with (
                tc.tile_pool(name="wvp", bufs=1) as wpool,
                tc.tile_pool(name="vin", bufs=2) as inpool,
                tc.tile_pool(name="vout", bufs=3) as outpool,
                tc.tile_pool(name="pjps2", bufs=2, space="PSUM") as pjps,
            ):
                wv_sb = wpool.tile([P, OC, EMB], f32r)
                nc.sync.dma_start(wv_sb[:], WvT.rearrange("(e p) o -> p e o", p=P))
                vT3 = vT.rearrange("(e p) t -> p e t", p=P)
                for tt in range(KC):  # 16 token tiles of 128
                    vin = inpool.tile([P, OC, P], f32r, tag="vin")
                    nc.sync.dma_start(vin[:], vT3[:, :, tt * P:(tt + 1) * P])
                    vp_out = outpool.tile([P, HP * 130], f32r, tag="vout", name="vp_out")
                    vp3 = vp_out.rearrange("p (h c) -> p h c", c=65)
                    for ot in range(2):  # halves of the 1024 out dims
                        ps = pjps.tile([P, N], f32, tag="pj", name="ps_v")
                        for ec in range(OC):
                            nc.tensor.matmul(
                                ps[:],
                                vin[:, ec, :],
                                wv_sb[:, ec, ot * N:(ot + 1) * N],
                                start=(ec == 0),
                                stop=False,
                            )
                        nc.tensor.matmul(
                            ps[:],
                            ones_sb[:],
                            bv_sb[:, ot * N:(ot + 1) * N],
                            start=False,
                            stop=True,
                        )
                        nc.vector.tensor_copy(
                            vp3[:, ot * 8:(ot + 1) * 8, 0:64],
                            ps.rearrange("p (h c) -> p h c", c=64),
                        )
                    nc.sync.dma_start(vp3[:, :, 64], ones_d[:, 0:HEADS])
                    nc.sync.dma_start(
                        Vp_d[tt].rearrange("p h c -> p (h c)"), vp_out[:]
                    )

            # ---------------- Q projection -> qt_big (SBUF) ----------------
            with (
                tc.tile_pool(name="wqp", bufs=1) as wpool,
                tc.tile_pool(name="qin", bufs=2) as inpool,
                tc.tile_pool(name="pjps3", bufs=2, space="PSUM") as pjps,
            ):
                wq_sb = wpool.tile([P, OC, EMB], f32r)
                nc.sync.dma_start(wq_sb[:], WqT.rearrange("(e p) o -> p e o", p=P))
                qT3 = qT.rearrange("(e p) t -> p e t", p=P)
                for tt in range(TQ // N):
                    qin = inpool.tile([P, OC, N], f32r, tag="qin")
                    nc.sync.dma_start(qin[:], qT3[:, :, tt * N:(tt + 1) * N])
                    for oc in range(OC):
                        ps = pjps.tile([P, N], f32, tag="pj", name="ps_q")
                        for ec in range(OC):
                            nc.tensor.matmul(
                                ps[:],
                                wq_sb[:, ec, oc * P:(oc + 1) * P],
                                qin[:, ec, :],
                                start=(ec == 0),
                                stop=(ec == OC - 1),
                            )
                        nc.vector.tensor_scalar_add(
                            qt_big[:, oc, tt * N:(tt + 1) * N], ps[:], bq_sb[:, oc:oc + 1]
                        )

            # ---------------- attention per head pair ----------------
            with (
                tc.tile_pool(name="ktp", bufs=2) as kt_pool,
                tc.tile_pool(name="vpp", bufs=2) as vp_pool,
                tc.tile_pool(name="xp", bufs=4) as x_pool,
                tc.tile_pool(name="rdp", bufs=2) as rd_pool,
                tc.tile_pool(name="scps", bufs=2, space="PSUM") as sc_psum,
                tc.tile_pool(name="ovps", bufs=2, space="PSUM") as ov_psum,
            ):
                for hp in range(HP):
                    ktp = kt_pool.tile([P, TK], f32r, tag="ktp", name="ktp")
                    nc.sync.dma_start(ktp[:], KT_d[hp])
                    vp_sb = vp_pool.tile([P, KC, 130], f32r, tag="vpp", name="vp_sb")
                    nc.sync.dma_start(vp_sb[:], Vp_d[:, :, hp, :].rearrange("k p c -> p k c"))
                    ovA = ov_psum.tile([65, TQ], f32, tag="ov", name="ovA")
                    ovB = ov_psum.tile([65, TQ], f32, tag="ov", name="ovB")
                    for kc in range(KC):
                        sA = sc_psum.tile([P, TQ], f32, tag="sc", name="sA")
                        sB = sc_psum.tile([P, TQ], f32, tag="sc", name="sB")
                        for qt in range(TQ // N):
                            qs = slice(qt * N, (qt + 1) * N)
                            nc.tensor.matmul(
                                sA[:, qs],
                                ktp[0:HD, kc * P:(kc + 1) * P],
                                qt_big[0:HD, hp, qs],
                                start=True,
                                stop=True,
                            )
                            nc.tensor.matmul(
                                sB[:, qs],
                                ktp[HD:P, kc * P:(kc + 1) * P],
                                qt_big[HD:P, hp, qs],
                                start=True,
                                stop=True,
                            )
                        xA = x_pool.tile([P, TQ], f32r, tag="x", name="xA")
                        xB = x_pool.tile([P, TQ], f32r, tag="x", name="xB")
                        nc.scalar.activation(xA[:], sA[:], EXP, scale=0.125)
                        nc.scalar.activation(xB[:], sB[:], EXP, scale=0.125)
                        for qt in range(TQ // N):
                            qs = slice(qt * N, (qt + 1) * N)
                            nc.tensor.matmul(
                                ovA[:, qs],
                                vp_sb[:, kc, 0:65],
                                xA[:, qs],
                                start=(kc == 0),
                                stop=(kc == KC - 1),
                            )
                            nc.tensor.matmul(
                                ovB[:, qs],
                                vp_sb[:, kc, 65:130],
                                xB[:, qs],
                                start=(kc == 0),
                                stop=(kc == KC - 1),
                            )
                    for j, ov in ((0, ovA), (1, ovB)):
                        h = 2 * hp + j
                        rd = rd_pool.tile([65, TQ], f32r, tag="rd", name="rd")
                        nc.vector.reciprocal(rd[64:65, :], ov[64:65, :])
                        RD = sc_psum.tile([HD, TQ], f32, tag="sc", name="RD")
                        for qt in range(TQ // N):
                            qs = slice(qt * N, (qt + 1) * N)
                            nc.tensor.matmul(
                                RD[:, qs],
                                ones64_sb[64:65, :],
                                rd[64:65, qs],
                                start=True,
                                stop=True,
                            )
                        RD_sb = rd_pool.tile([HD, TQ], f32r, tag="rdsb", name="RD_sb")
                        nc.vector.tensor_copy(RD_sb[:], RD[:])
                        nc.vector.tensor_mul(resT[h][:], ov[0:HD, :], RD_sb[:])

            # ---------------- output projection ----------------
            with (
                tc.tile_pool(name="wop", bufs=1) as wpool,
                tc.tile_pool(name="oout", bufs=3) as outpool,
                tc.tile_pool(name="ops", bufs=2, space="PSUM") as ops,
            ):
                wo_sb = wpool.tile([HD, HEADS, EMB], f32r)
                nc.sync.dma_start(wo_sb[:], WoT.rearrange("(h p) o -> p h o", p=HD))
                for qt8 in range(TQ // P):
                    for ot in range(2):
                        os_ = slice(ot * N, (ot + 1) * N)
                        ps = ops.tile([P, N], f32, tag="o", name="ps_o")
                        for h in range(HEADS):
                            nc.tensor.matmul(
                                ps[:],
                                resT[h][:, qt8 * P:(qt8 + 1) * P],
                                wo_sb[:, h, os_],
                                start=(h == 0),
                                stop=False,
                            )
                        nc.tensor.matmul(
                            ps[:], ones_sb[:], bo_sb[:, os_], start=False, stop=True
                        )
                        o_sb = outpool.tile([P, N], f32, tag="oout", name="o_sb")
                        nc.vector.tensor_copy(o_sb[:], ps[:])
                        nc.sync.dma_start(out[qt8 * P:(qt8 + 1) * P, os_], o_sb[:])

    nc.compile()
    return nc


def _get_nc():
    if "nc" not in _CACHE:
        _CACHE["nc"] = _build()
    return _CACHE["nc"]


def make_in_maps(q, k, v, Wq, bq, Wk, bk, Wv, bv, Wo, bo):
    """Host-side sharding: per-core input dicts (with pre-transposed layouts)."""
    f = np.float32
    WqT = np.ascontiguousarray(np.asarray(Wq, f).T)
    WkT = np.ascontiguousarray(np.asarray(Wk, f).T)
    WvT = np.ascontiguousarray(np.asarray(Wv, f).T)
    WoT = np.ascontiguousarray(np.asarray(Wo, f).T)
    bq2 = np.ascontiguousarray(np.asarray(bq, f).reshape(OC, P))
    bk2 = np.ascontiguousarray(np.asarray(bk, f).reshape(OC, P))
    bv2 = np.ascontiguousarray(np.asarray(bv, f).reshape(1, EMB))
    bo2 = np.ascontiguousarray(np.asarray(bo, f).reshape(1, EMB))
    qT_b = [np.ascontiguousarray(np.asarray(q[b], f).T) for b in range(B)]
    kT_b = [np.ascontiguousarray(np.asarray(k[b], f).T) for b in range(B)]
    vT_b = [np.ascontiguousarray(np.asarray(v[b], f).T) for b in range(B)]
    in_maps = []
    for c in range(NCORES):
        b, half = c // 2, c % 2
        in_maps.append({
            "qT": np.ascontiguousarray(qT_b[b][:, half * TQ:(half + 1) * TQ]),
            "kT": kT_b[b],
            "vT": vT_b[b],
            "WqT": WqT, "WkT": WkT, "WvT": WvT, "WoT": WoT,
            "bq": bq2, "bk": bk2, "bv": bv2, "bo": bo2,
            "ones_d": np.ones((P, P), f),
        })
    return in_maps


def kernel(q, k, v, Wq, bq, Wk, bk, Wv, bv, Wo, bo):
    from concourse.bass_utils import run_bass_kernel_spmd

    nc = _get_nc()
    in_maps = make_in_maps(q, k, v, Wq, bq, Wk, bk, Wv, bv, Wo, bo)
    res = run_bass_kernel_spmd(nc, in_maps, core_ids=list(range(NCORES)))
    out = np.empty((B, S, EMB), np.float32)
    for c in range(NCORES):
        b, half = c // 2, c % 2
        out[b, half * TQ:(half + 1) * TQ, :] = res.results[c]["out"]
    return out
